# revision 24
# baseline (speedup 1.0000x reference)
"""Multi-head self-attention (post-softmax gauss reweight variant) on 8 TRN2 cores.

Sharding: core c handles batch b = c//2 and query-row half r = c%2 (512 rows),
all 16 heads. No cross-core communication.

Math (per batch b):
  q = (query @ Wq + bq) / 8 ;  k = key @ Wk + bk ;  v = value @ Wv + bv
  softmax -> gauss reweight -> renormalize collapses to a single
  normalization:  w_qk = exp(s_qk + lnG_k) / sum_k exp(s_qk + lnG_k)
  where lnG_k = ln(gauss_k + 1e-10) + (mask_k==0 ? -1e9 : 0); the softmax
  denominator cancels against the renormalization.
  out = (w @ v) @ Wo + bo

Device layout: everything is computed in "transposed" orientation
(dims on partitions, sequence on free axis):
  qT = Wq^T @ query^T  (per 128-dim tile, bf16 PE matmuls, fp32 PSUM)
  kT likewise;  v in normal [kpos, dh] orientation (lhsT = value^T).
  sT[kpos, q] = kT' . qT' per head (K=64, two heads row-packed in the PE,
  both heads' scores land in one [128,1024] 2-bank PSUM tile).
  p = exp(sT + lnG) on ACT in one [128,1024] op (bias is per-partition in
  this orientation; the gauss/mask factor rides the exp bias for free).
  PV: lhsT = [v_h | 1] (M=65) -> psC[0:64]=ctx^T, psC[64]=denominator.
  Normalize: reciprocal_approx_fast on the den row, gpsimd
  partition_broadcast to 64 partitions, DVE multiply (out to bf16 ctx).
  out = ctx^T.T @ Wo + bo (rank-1 ones x bias for the free-axis biases).
"""

import os
import sys
import types

sys.path.insert(0, "/opt/trn_rl_repo")

import numpy as np

# The agent image's antenv package lacks axon_hooks, so trn_boot's NTFF hook
# registration silently degrades. Recreate the module so
# run_bass_kernel_spmd(trace=True) can profile (used by test.py; harmless
# otherwise).
try:
    import antenv

    if "antenv.axon_hooks" not in sys.modules:
        _hooks_mod = types.ModuleType("antenv.axon_hooks")
        _hooks_mod._hook = None
        _hooks_mod.set_axon_ntff_profile_hook = lambda h: setattr(
            _hooks_mod, "_hook", h
        )
        _hooks_mod.get_axon_ntff_profile_hook = lambda: _hooks_mod._hook
        sys.modules["antenv.axon_hooks"] = _hooks_mod
        antenv.axon_hooks = _hooks_mod
        try:
            from trn_agent_boot.trn_boot import _ntff_profile_via_ctypes

            _hook = _ntff_profile_via_ctypes("/opt/axon/libaxon_pjrt.so")
            if _hook is not None:
                _hooks_mod.set_axon_ntff_profile_hook(_hook)
        except Exception:
            pass
except Exception:
    pass

import concourse.bass as bass
import concourse.mybir as mybir
import concourse.tile as tile
from concourse import bacc
from concourse import bass_utils

BS, SEQ, DIM, H = 4, 1024, 1024, 16
DH = DIM // H  # 64
QH = SEQ // 2  # 512 rows of q per core
N_CORES = 8
KT = DIM // 128  # 8 contraction tiles
PT = SEQ // 128  # 8 kpos tiles
NPAIR = H // 2  # 8 head pairs

F32 = mybir.dt.float32
BF16 = mybir.dt.bfloat16
I32 = mybir.dt.int32
AF = mybir.ActivationFunctionType

_CACHED = {}
LAST_RESULT = None


def _build():
    nc = bacc.Bacc("TRN2", target_bir_lowering=False, debug=False, num_devices=N_CORES)

    qT = nc.dram_tensor("qT", [DIM, QH], F32, kind="ExternalInput").ap()
    kT = nc.dram_tensor("kT", [DIM, SEQ], F32, kind="ExternalInput").ap()
    vT = nc.dram_tensor("vT", [DIM, SEQ], F32, kind="ExternalInput").ap()
    Wq = nc.dram_tensor("Wq", [DIM, DIM], F32, kind="ExternalInput").ap()
    Wk = nc.dram_tensor("Wk", [DIM, DIM], F32, kind="ExternalInput").ap()
    Wv = nc.dram_tensor("Wv", [DIM, DIM], F32, kind="ExternalInput").ap()
    Wo = nc.dram_tensor("Wo", [DIM, DIM], F32, kind="ExternalInput").ap()
    bq = nc.dram_tensor("bq", [DIM], F32, kind="ExternalInput").ap()
    bk = nc.dram_tensor("bk", [DIM], F32, kind="ExternalInput").ap()
    bv = nc.dram_tensor("bv", [DIM], F32, kind="ExternalInput").ap()
    bo = nc.dram_tensor("bo", [DIM], F32, kind="ExternalInput").ap()
    gauss = nc.dram_tensor("gauss", [SEQ], F32, kind="ExternalInput").ap()
    mask = nc.dram_tensor("mask", [SEQ], I32, kind="ExternalInput").ap()
    out = nc.dram_tensor("out", [QH, DIM], F32, kind="ExternalOutput").ap()

    with tile.TileContext(nc, pool_alloc_mode="queue") as tc:
        with (
            tc.tile_pool(name="const", bufs=1) as constp,
            tc.tile_pool(name="small", bufs=1) as smallp,
            tc.tile_pool(name="big", bufs=1) as bigp,
            tc.tile_pool(name="qtp", bufs=8) as qtpp,
            tc.tile_pool(name="ktp", bufs=8) as ktpp,
            tc.tile_pool(name="vsb", bufs=8) as vsbp,
            tc.tile_pool(name="psb", bufs=4) as psbp,
            tc.tile_pool(name="ctx", bufs=8) as ctxp,
            tc.tile_pool(name="norm", bufs=3) as normp,
            tc.tile_pool(name="osb", bufs=2) as osbp,
            tc.tile_pool(name="acc", bufs=2, space="PSUM") as accp,
            tc.tile_pool(name="sps", bufs=2, space="PSUM") as spsp,
            tc.tile_pool(name="cps", bufs=2, space="PSUM") as cpsp,
        ):
            # ---- constants / small tensors ----
            ones_f = constp.tile([128, 128], F32)
            nc.gpsimd.memset(ones_f[:], 1.0)
            ones = constp.tile([128, 128], BF16)
            nc.vector.tensor_copy(ones[:], ones_f[:])

            g_sb = smallp.tile([128, PT], F32)
            nc.sync.dma_start(out=g_sb[:], in_=gauss.rearrange("(t p) -> p t", p=128))
            m_i = smallp.tile([128, PT], I32)
            nc.sync.dma_start(out=m_i[:], in_=mask.rearrange("(t p) -> p t", p=128))
            m_f = smallp.tile([128, PT], F32)
            nc.vector.tensor_copy(m_f[:], m_i[:])
            # lnG = ln(gauss + 1e-10) + (mask - 1) * 1e9
            eps_t = smallp.tile([128, 1], F32)
            nc.gpsimd.memset(eps_t[:], 1e-10)
            lnG = smallp.tile([128, PT], F32)
            nc.scalar.activation(lnG[:], g_sb[:], AF.Ln, bias=eps_t[:, 0:1], scale=1.0)
            pen = smallp.tile([128, PT], F32)
            nc.vector.tensor_scalar(
                pen[:], m_f[:], 1e9, -1e9, mybir.AluOpType.mult, mybir.AluOpType.add
            )
            nc.vector.tensor_add(lnG[:], lnG[:], pen[:])

            bqs = smallp.tile([128, KT], F32)
            nc.sync.dma_start(out=bqs[:], in_=bq.rearrange("(t p) -> p t", p=128))
            nc.vector.tensor_scalar_mul(bqs[:], bqs[:], 0.125)
            bks = smallp.tile([128, KT], F32)
            nc.sync.dma_start(out=bks[:], in_=bk.rearrange("(t p) -> p t", p=128))
            bv_f = smallp.tile([1, DIM], F32)
            nc.sync.dma_start(out=bv_f[:], in_=bv.rearrange("(a d) -> a d", a=1))
            bv_sb = smallp.tile([1, DIM], BF16)
            nc.vector.tensor_copy(bv_sb[:], bv_f[:])
            bo_f = smallp.tile([1, DIM], F32)
            nc.sync.dma_start(out=bo_f[:], in_=bo.rearrange("(a d) -> a d", a=1))
            bo_sb = smallp.tile([1, DIM], BF16)
            nc.vector.tensor_copy(bo_sb[:], bo_f[:])

            # Bulk load: SWDGE cast-DMAs (fp32 DRAM -> bf16 SBUF, cast
            # inline in the DMA datapath), split into 2MB chunks so the
            # first projection can start after ~4MB instead of ~10MB.
            # Everything stays resident in SBUF for the whole kernel.
            def alloc_big(cols, name):
                return bigp.tile([128, KT * cols], BF16, tag=name, name=name)

            def load_chunk(bt, src_ap, cols, h):
                # chunk h covers contraction tiles t in [4h, 4h+4)
                nc.gpsimd.dma_start(
                    out=bt[:, 4 * h * cols : 4 * (h + 1) * cols].rearrange(
                        "p (t d) -> p t d", d=cols
                    ),
                    in_=src_ap[512 * h : 512 * (h + 1), :].rearrange(
                        "(t p) d -> p t d", p=128
                    ),
                )

            qTb = alloc_big(QH, "qTb")
            wqb = alloc_big(DIM, "wqb")
            wkb = alloc_big(DIM, "wkb")
            ktb = alloc_big(SEQ, "ktb")
            vtb = alloc_big(SEQ, "vtb")
            wvb = alloc_big(DIM, "wvb")
            wob = alloc_big(DIM, "wob")

            qTp = [
                qtpp.tile([128, QH], BF16, tag="qtp", name=f"qTp{j}")
                for j in range(KT)
            ]
            kTp = [
                ktpp.tile([128, SEQ], BF16, tag="ktp", name=f"kTp{j}")
                for j in range(KT)
            ]
            v_sb = []
            for m in range(PT):
                vm = vsbp.tile([128, H * (DH + 1)], BF16, tag="vsb", name=f"v{m}")
                vv = vm[:].rearrange("p (h c) -> p h c", c=DH + 1)
                nc.vector.tensor_copy(vv[:, :, DH : DH + 1], ones_f[:, 0:H])
                v_sb.append(vm)
            ctx_sb = [
                ctxp.tile([128, QH], BF16, tag="ctx", name=f"ctx{p}")
                for p in range(NPAIR)
            ]

            def do_qproj(j):
                ps = accp.tile([128, QH], F32, tag="ps", name=f"psq{j}")
                for t in range(KT):
                    nc.tensor.matmul(
                        ps[:],
                        wqb[:, DIM * t + 128 * j : DIM * t + 128 * (j + 1)],
                        qTb[:, QH * t : QH * (t + 1)],
                        start=(t == 0),
                        stop=(t == KT - 1),
                    )
                nc.vector.tensor_scalar(
                    qTp[j][:], ps[:], 0.125, bqs[:, j : j + 1],
                    mybir.AluOpType.mult, mybir.AluOpType.add,
                )

            def do_kproj(j):
                for n in range(2):
                    ps = accp.tile([128, 512], F32, tag="ps", name=f"psk{j}_{n}")
                    for t in range(KT):
                        nc.tensor.matmul(
                            ps[:],
                            wkb[:, DIM * t + 128 * j : DIM * t + 128 * (j + 1)],
                            ktb[:, SEQ * t + 512 * n : SEQ * t + 512 * (n + 1)],
                            start=(t == 0),
                            stop=(t == KT - 1),
                        )
                    nc.vector.tensor_scalar(
                        kTp[j][:, 512 * n : 512 * (n + 1)],
                        ps[:],
                        bks[:, j : j + 1],
                        None,
                        mybir.AluOpType.add,
                    )

            def do_vproj(n, m):
                ps = accp.tile([128, 512], F32, tag="ps", name=f"psv{n}_{m}")
                for t in range(KT):
                    nc.tensor.matmul(
                        ps[:],
                        vtb[:, SEQ * t + 128 * m : SEQ * t + 128 * (m + 1)],
                        wvb[:, DIM * t + 512 * n : DIM * t + 512 * (n + 1)],
                        start=(t == 0),
                        stop=False,
                    )
                nc.tensor.matmul(
                    ps[:],
                    ones[0:1, 0:128],
                    bv_sb[0:1, 512 * n : 512 * (n + 1)],
                    start=False,
                    stop=True,
                )
                vv = v_sb[m][:].rearrange("p (h c) -> p h c", c=DH + 1)
                nc.vector.tensor_copy(
                    vv[:, 8 * n : 8 * (n + 1), 0:DH],
                    ps[:].rearrange("p (h c) -> p h c", c=DH),
                )

            def do_pair(p):
                psC_A = cpsp.tile([65, QH], F32, tag="cps", name=f"psCA{p}")
                psC_B = cpsp.tile([65, QH], F32, tag="cps", name=f"psCB{p}")
                for t in range(PT):
                    psS = spsp.tile([128, 2 * QH], F32, tag="sps", name=f"psS{p}_{t}")
                    nc.tensor.matmul(
                        psS[:, 0:QH],
                        kTp[p][0:64, 128 * t : 128 * (t + 1)],
                        qTp[p][0:64, :],
                        tile_position=(0, 0),
                    )
                    nc.tensor.matmul(
                        psS[:, QH : 2 * QH],
                        kTp[p][64:128, 128 * t : 128 * (t + 1)],
                        qTp[p][64:128, :],
                        tile_position=(64, 0),
                    )
                    pAB = psbp.tile([128, 2 * QH], BF16, tag="psb", name=f"p{p}_{t}")
                    nc.scalar.activation(
                        pAB[:], psS[:], AF.Exp, bias=lnG[:, t : t + 1], scale=1.0
                    )
                    vv = v_sb[t][:].rearrange("p (h c) -> p h c", c=DH + 1)
                    nc.tensor.matmul(
                        psC_A[:],
                        vv[:, 2 * p, :],
                        pAB[:, 0:QH],
                        start=(t == 0),
                        stop=(t == PT - 1),
                    )
                    nc.tensor.matmul(
                        psC_B[:],
                        vv[:, 2 * p + 1, :],
                        pAB[:, QH : 2 * QH],
                        start=(t == 0),
                        stop=(t == PT - 1),
                    )
                # normalize: ctx rows 0:64 divided by den row 64
                denA = normp.tile([1, QH], F32, tag="den", name=f"denA{p}")
                denB = normp.tile([1, QH], F32, tag="den", name=f"denB{p}")
                nc.vector.tensor_copy(denA[:], psC_A[64:65, :])
                nc.vector.tensor_copy(denB[:], psC_B[64:65, :])
                recA = normp.tile([1, QH], F32, tag="rec", name=f"recA{p}")
                recB = normp.tile([1, QH], F32, tag="rec", name=f"recB{p}")
                nc.vector.reciprocal_approx_fast(recA[:], denA[:])
                nc.vector.reciprocal_approx_fast(recB[:], denB[:])
                bcA = normp.tile([64, QH], F32, tag="bc", name=f"bcA{p}")
                bcB = normp.tile([64, QH], F32, tag="bc", name=f"bcB{p}")
                nc.gpsimd.partition_broadcast(bcA[:], recA[0:1, :])
                nc.gpsimd.partition_broadcast(bcB[:], recB[0:1, :])
                nc.vector.tensor_mul(ctx_sb[p][0:64, :], psC_A[0:64, :], bcA[:])
                nc.vector.tensor_mul(ctx_sb[p][64:128, :], psC_B[0:64, :], bcB[:])

            def do_oproj(n, m):
                ps = accp.tile([128, 512], F32, tag="ps", name=f"pso{n}_{m}")
                for t in range(KT):
                    nc.tensor.matmul(
                        ps[:],
                        ctx_sb[t][:, 128 * m : 128 * (m + 1)],
                        wob[:, DIM * t + 512 * n : DIM * t + 512 * (n + 1)],
                        start=(t == 0),
                        stop=False,
                    )
                nc.tensor.matmul(
                    ps[:],
                    ones[0:1, 0:128],
                    bo_sb[0:1, 512 * n : 512 * (n + 1)],
                    start=False,
                    stop=True,
                )
                os_t = osbp.tile([128, 512], F32, tag="osb", name=f"os{n}_{m}")
                nc.vector.tensor_copy(os_t[:], ps[:])
                nc.sync.dma_start(
                    out=out[128 * m : 128 * (m + 1), 512 * n : 512 * (n + 1)],
                    in_=os_t[:],
                )

            # ---- emission schedule: DMA chunks early, attention pairs
            # interleaved with remaining projection work so the ACT-bound
            # exp stream overlaps PE-bound projection matmuls.
            load_chunk(qTb, qT, QH, 0)
            load_chunk(wqb, Wq, DIM, 0)
            load_chunk(qTb, qT, QH, 1)
            load_chunk(wqb, Wq, DIM, 1)
            load_chunk(wkb, Wk, DIM, 0)
            load_chunk(ktb, kT, SEQ, 0)
            load_chunk(wkb, Wk, DIM, 1)
            load_chunk(ktb, kT, SEQ, 1)
            load_chunk(vtb, vT, SEQ, 0)
            load_chunk(wvb, Wv, DIM, 0)
            load_chunk(vtb, vT, SEQ, 1)
            load_chunk(wvb, Wv, DIM, 1)
            load_chunk(wob, Wo, DIM, 0)
            load_chunk(wob, Wo, DIM, 1)

            for j in range(KT):
                do_qproj(j)
            for j in range(4):
                do_kproj(j)
            for m in range(PT):
                do_vproj(0, m)
            do_pair(0)
            do_kproj(4)
            do_vproj(1, 0)
            do_pair(1)
            do_kproj(5)
            do_vproj(1, 1)
            do_pair(2)
            do_kproj(6)
            do_vproj(1, 2)
            do_vproj(1, 3)
            do_pair(3)
            do_kproj(7)
            do_vproj(1, 4)
            do_vproj(1, 5)
            do_vproj(1, 6)
            do_vproj(1, 7)
            do_pair(4)
            do_pair(5)
            do_pair(6)
            do_pair(7)
            for n in range(2):
                for m in range(QH // 128):
                    do_oproj(n, m)

    nc.compile()
    return nc


def kernel(
    query, key, value, mask, gauss_weight, Wq, bq, Wk, bk, Wv, bv, Wo, bo
) -> np.ndarray:
    global LAST_RESULT
    if "nc" not in _CACHED:
        _CACHED["nc"] = _build()
    nc = _CACHED["nc"]

    query = np.asarray(query, dtype=np.float32)
    key = np.asarray(key, dtype=np.float32)
    value = np.asarray(value, dtype=np.float32)
    mask = np.asarray(mask, dtype=np.int32)
    gauss_weight = np.asarray(gauss_weight, dtype=np.float32)
    shared = {
        "Wq": np.ascontiguousarray(Wq, dtype=np.float32),
        "Wk": np.ascontiguousarray(Wk, dtype=np.float32),
        "Wv": np.ascontiguousarray(Wv, dtype=np.float32),
        "Wo": np.ascontiguousarray(Wo, dtype=np.float32),
        "bq": np.ascontiguousarray(bq, dtype=np.float32),
        "bk": np.ascontiguousarray(bk, dtype=np.float32),
        "bv": np.ascontiguousarray(bv, dtype=np.float32),
        "bo": np.ascontiguousarray(bo, dtype=np.float32),
    }

    in_maps = []
    for c in range(N_CORES):
        b, r = c // 2, c % 2
        qTb = np.ascontiguousarray(query[b].T[:, QH * r : QH * (r + 1)])
        in_maps.append(
            {
                "qT": qTb,
                "kT": np.ascontiguousarray(key[b].T),
                "vT": np.ascontiguousarray(value[b].T),
                "gauss": np.ascontiguousarray(gauss_weight[b]),
                "mask": np.ascontiguousarray(mask[b]),
                **shared,
            }
        )

    res = None
    last_exc = None
    for _attempt in range(3):
        try:
            res = bass_utils.run_bass_kernel_spmd(
                nc, in_maps, core_ids=list(range(N_CORES))
            )
            break
        except Exception as e:  # transient NRT_EXEC_UNIT faults on first exec
            last_exc = e
    if res is None:
        raise last_exc
    LAST_RESULT = res

    output = np.empty((BS, SEQ, DIM), dtype=np.float32)
    for c in range(N_CORES):
        b, r = c // 2, c % 2
        output[b, QH * r : QH * (r + 1), :] = res.results[c]["out"]
    return output


# revision 25
# speedup vs baseline: 1.0849x; 1.0849x over previous
"""Multi-head self-attention (post-softmax gauss reweight variant) on 8 TRN2 cores.

Sharding: core c handles batch b = c//2 and query-row half r = c%2 (512 rows),
all 16 heads. No cross-core communication.

Math (per batch b):
  q = (query @ Wq + bq) / 8 ;  k = key @ Wk + bk ;  v = value @ Wv + bv
  softmax -> gauss reweight -> renormalize collapses to a single
  normalization:  w_qk = exp(s_qk + lnG_k) / sum_k exp(s_qk + lnG_k)
  where lnG_k = ln(gauss_k + 1e-10) + (mask_k==0 ? -1e9 : 0); the softmax
  denominator cancels against the renormalization.
  out = (w @ v) @ Wo + bo

Device layout: everything is computed in "transposed" orientation
(dims on partitions, sequence on free axis):
  qT = Wq^T @ query^T  (per 128-dim tile, bf16 PE matmuls, fp32 PSUM)
  kT likewise;  v in normal [kpos, dh] orientation (lhsT = value^T).
  sT[kpos, q] = kT' . qT' per head (K=64, two heads row-packed in the PE,
  both heads' scores land in one [128,1024] 2-bank PSUM tile).
  p = exp(sT + lnG) on ACT in one [128,1024] op (bias is per-partition in
  this orientation; the gauss/mask factor rides the exp bias for free).
  PV: lhsT = [v_h | 1] (M=65) -> psC[0:64]=ctx^T, psC[64]=denominator.
  Normalize: reciprocal_approx_fast on the den row, gpsimd
  partition_broadcast to 64 partitions, DVE multiply (out to bf16 ctx).
  out = ctx^T.T @ Wo + bo (rank-1 ones x bias for the free-axis biases).
"""

import os
import sys
import types

sys.path.insert(0, "/opt/trn_rl_repo")

import numpy as np

# The agent image's antenv package lacks axon_hooks, so trn_boot's NTFF hook
# registration silently degrades. Recreate the module so
# run_bass_kernel_spmd(trace=True) can profile (used by test.py; harmless
# otherwise).
try:
    import antenv

    if "antenv.axon_hooks" not in sys.modules:
        _hooks_mod = types.ModuleType("antenv.axon_hooks")
        _hooks_mod._hook = None
        _hooks_mod.set_axon_ntff_profile_hook = lambda h: setattr(
            _hooks_mod, "_hook", h
        )
        _hooks_mod.get_axon_ntff_profile_hook = lambda: _hooks_mod._hook
        sys.modules["antenv.axon_hooks"] = _hooks_mod
        antenv.axon_hooks = _hooks_mod
        try:
            from trn_agent_boot.trn_boot import _ntff_profile_via_ctypes

            _hook = _ntff_profile_via_ctypes("/opt/axon/libaxon_pjrt.so")
            if _hook is not None:
                _hooks_mod.set_axon_ntff_profile_hook(_hook)
        except Exception:
            pass
except Exception:
    pass

import concourse.bass as bass
import concourse.mybir as mybir
import concourse.tile as tile
from concourse import bacc
from concourse import bass_utils

BS, SEQ, DIM, H = 4, 1024, 1024, 16
DH = DIM // H  # 64
QH = SEQ // 2  # 512 rows of q per core
N_CORES = 8
KT = DIM // 128  # 8 contraction tiles
PT = SEQ // 128  # 8 kpos tiles
NPAIR = H // 2  # 8 head pairs

F32 = mybir.dt.float32
BF16 = mybir.dt.bfloat16
I32 = mybir.dt.int32
AF = mybir.ActivationFunctionType

_CACHED = {}
LAST_RESULT = None


def _build():
    nc = bacc.Bacc("TRN2", target_bir_lowering=False, debug=False, num_devices=N_CORES)

    qT = nc.dram_tensor("qT", [DIM, QH], F32, kind="ExternalInput").ap()
    kT = nc.dram_tensor("kT", [DIM, SEQ], F32, kind="ExternalInput").ap()
    vT = nc.dram_tensor("vT", [DIM, SEQ], F32, kind="ExternalInput").ap()
    Wq = nc.dram_tensor("Wq", [DIM, DIM], F32, kind="ExternalInput").ap()
    Wk = nc.dram_tensor("Wk", [DIM, DIM], F32, kind="ExternalInput").ap()
    Wv = nc.dram_tensor("Wv", [DIM, DIM], F32, kind="ExternalInput").ap()
    Wo = nc.dram_tensor("Wo", [DIM, DIM], F32, kind="ExternalInput").ap()
    bq = nc.dram_tensor("bq", [DIM], F32, kind="ExternalInput").ap()
    bk = nc.dram_tensor("bk", [DIM], F32, kind="ExternalInput").ap()
    bv = nc.dram_tensor("bv", [DIM], F32, kind="ExternalInput").ap()
    bo = nc.dram_tensor("bo", [DIM], F32, kind="ExternalInput").ap()
    gauss = nc.dram_tensor("gauss", [SEQ], F32, kind="ExternalInput").ap()
    mask = nc.dram_tensor("mask", [SEQ], I32, kind="ExternalInput").ap()
    out = nc.dram_tensor("out", [QH, DIM], F32, kind="ExternalOutput").ap()

    with tile.TileContext(nc) as tc:
        with (
            tc.tile_pool(name="const", bufs=1) as constp,
            tc.tile_pool(name="small", bufs=1) as smallp,
            tc.tile_pool(name="big", bufs=1) as bigp,
            tc.tile_pool(name="qtp", bufs=8) as qtpp,
            tc.tile_pool(name="ktp", bufs=8) as ktpp,
            tc.tile_pool(name="vsb", bufs=8) as vsbp,
            tc.tile_pool(name="psb", bufs=4) as psbp,
            tc.tile_pool(name="ctx", bufs=8) as ctxp,
            tc.tile_pool(name="norm", bufs=3) as normp,
            tc.tile_pool(name="osb", bufs=2) as osbp,
            tc.tile_pool(name="acc", bufs=2, space="PSUM") as accp,
            tc.tile_pool(name="sps", bufs=2, space="PSUM") as spsp,
            tc.tile_pool(name="cps", bufs=2, space="PSUM") as cpsp,
        ):
            # ---- constants / small tensors ----
            ones_f = constp.tile([128, 128], F32)
            nc.gpsimd.memset(ones_f[:], 1.0)
            ones = constp.tile([128, 128], BF16)
            nc.vector.tensor_copy(ones[:], ones_f[:])

            g_sb = smallp.tile([128, PT], F32)
            nc.sync.dma_start(out=g_sb[:], in_=gauss.rearrange("(t p) -> p t", p=128))
            m_i = smallp.tile([128, PT], I32)
            nc.sync.dma_start(out=m_i[:], in_=mask.rearrange("(t p) -> p t", p=128))
            m_f = smallp.tile([128, PT], F32)
            nc.vector.tensor_copy(m_f[:], m_i[:])
            # lnG = ln(gauss + 1e-10) + (mask - 1) * 1e9
            eps_t = smallp.tile([128, 1], F32)
            nc.gpsimd.memset(eps_t[:], 1e-10)
            lnG = smallp.tile([128, PT], F32)
            nc.scalar.activation(lnG[:], g_sb[:], AF.Ln, bias=eps_t[:, 0:1], scale=1.0)
            pen = smallp.tile([128, PT], F32)
            nc.vector.tensor_scalar(
                pen[:], m_f[:], 1e9, -1e9, mybir.AluOpType.mult, mybir.AluOpType.add
            )
            nc.vector.tensor_add(lnG[:], lnG[:], pen[:])

            bqs = smallp.tile([128, KT], F32)
            nc.sync.dma_start(out=bqs[:], in_=bq.rearrange("(t p) -> p t", p=128))
            nc.vector.tensor_scalar_mul(bqs[:], bqs[:], 0.125)
            bks = smallp.tile([128, KT], F32)
            nc.sync.dma_start(out=bks[:], in_=bk.rearrange("(t p) -> p t", p=128))
            bv_f = smallp.tile([1, DIM], F32)
            nc.sync.dma_start(out=bv_f[:], in_=bv.rearrange("(a d) -> a d", a=1))
            bv_sb = smallp.tile([1, DIM], BF16)
            nc.vector.tensor_copy(bv_sb[:], bv_f[:])
            bo_f = smallp.tile([1, DIM], F32)
            nc.sync.dma_start(out=bo_f[:], in_=bo.rearrange("(a d) -> a d", a=1))
            bo_sb = smallp.tile([1, DIM], BF16)
            nc.vector.tensor_copy(bo_sb[:], bo_f[:])

            # Bulk load: SWDGE cast-DMAs (fp32 DRAM -> bf16 SBUF, cast
            # inline in the DMA datapath), split into 2MB chunks so the
            # first projection can start after ~4MB instead of ~10MB.
            # Everything stays resident in SBUF for the whole kernel.
            def alloc_big(cols, name):
                return bigp.tile([128, KT * cols], BF16, tag=name, name=name)

            def load_chunk(bt, src_ap, cols, h):
                # chunk h covers contraction tiles t in [4h, 4h+4)
                nc.gpsimd.dma_start(
                    out=bt[:, 4 * h * cols : 4 * (h + 1) * cols].rearrange(
                        "p (t d) -> p t d", d=cols
                    ),
                    in_=src_ap[512 * h : 512 * (h + 1), :].rearrange(
                        "(t p) d -> p t d", p=128
                    ),
                )

            qTb = alloc_big(QH, "qTb")
            wqb = alloc_big(DIM, "wqb")
            wkb = alloc_big(DIM, "wkb")
            ktb = alloc_big(SEQ, "ktb")
            vtb = alloc_big(SEQ, "vtb")
            wvb = alloc_big(DIM, "wvb")
            wob = alloc_big(DIM, "wob")

            qTp = [
                qtpp.tile([128, QH], BF16, tag="qtp", name=f"qTp{j}")
                for j in range(KT)
            ]
            kTp = [
                ktpp.tile([128, SEQ], BF16, tag="ktp", name=f"kTp{j}")
                for j in range(KT)
            ]
            v_sb = []
            for m in range(PT):
                vm = vsbp.tile([128, H * (DH + 1)], BF16, tag="vsb", name=f"v{m}")
                vv = vm[:].rearrange("p (h c) -> p h c", c=DH + 1)
                nc.vector.tensor_copy(vv[:, :, DH : DH + 1], ones_f[:, 0:H])
                v_sb.append(vm)
            ctx_sb = [
                ctxp.tile([128, QH], BF16, tag="ctx", name=f"ctx{p}")
                for p in range(NPAIR)
            ]

            def do_qproj(j):
                ps = accp.tile([128, QH], F32, tag="ps", name=f"psq{j}")
                for t in range(KT):
                    nc.tensor.matmul(
                        ps[:],
                        wqb[:, DIM * t + 128 * j : DIM * t + 128 * (j + 1)],
                        qTb[:, QH * t : QH * (t + 1)],
                        start=(t == 0),
                        stop=(t == KT - 1),
                    )
                nc.vector.tensor_scalar(
                    qTp[j][:], ps[:], 0.125, bqs[:, j : j + 1],
                    mybir.AluOpType.mult, mybir.AluOpType.add,
                )

            def do_kproj(j):
                for n in range(2):
                    ps = accp.tile([128, 512], F32, tag="ps", name=f"psk{j}_{n}")
                    for t in range(KT):
                        nc.tensor.matmul(
                            ps[:],
                            wkb[:, DIM * t + 128 * j : DIM * t + 128 * (j + 1)],
                            ktb[:, SEQ * t + 512 * n : SEQ * t + 512 * (n + 1)],
                            start=(t == 0),
                            stop=(t == KT - 1),
                        )
                    nc.vector.tensor_scalar(
                        kTp[j][:, 512 * n : 512 * (n + 1)],
                        ps[:],
                        bks[:, j : j + 1],
                        None,
                        mybir.AluOpType.add,
                    )

            def do_vproj(n, m):
                ps = accp.tile([128, 512], F32, tag="ps", name=f"psv{n}_{m}")
                for t in range(KT):
                    nc.tensor.matmul(
                        ps[:],
                        vtb[:, SEQ * t + 128 * m : SEQ * t + 128 * (m + 1)],
                        wvb[:, DIM * t + 512 * n : DIM * t + 512 * (n + 1)],
                        start=(t == 0),
                        stop=False,
                    )
                nc.tensor.matmul(
                    ps[:],
                    ones[0:1, 0:128],
                    bv_sb[0:1, 512 * n : 512 * (n + 1)],
                    start=False,
                    stop=True,
                )
                vv = v_sb[m][:].rearrange("p (h c) -> p h c", c=DH + 1)
                nc.vector.tensor_copy(
                    vv[:, 8 * n : 8 * (n + 1), 0:DH],
                    ps[:].rearrange("p (h c) -> p h c", c=DH),
                )

            def do_pair(p):
                psC_A = cpsp.tile([65, QH], F32, tag="cps", name=f"psCA{p}")
                psC_B = cpsp.tile([65, QH], F32, tag="cps", name=f"psCB{p}")
                for t in range(PT):
                    psS = spsp.tile([128, 2 * QH], F32, tag="sps", name=f"psS{p}_{t}")
                    nc.tensor.matmul(
                        psS[:, 0:QH],
                        kTp[p][0:64, 128 * t : 128 * (t + 1)],
                        qTp[p][0:64, :],
                        tile_position=(0, 0),
                    )
                    nc.tensor.matmul(
                        psS[:, QH : 2 * QH],
                        kTp[p][64:128, 128 * t : 128 * (t + 1)],
                        qTp[p][64:128, :],
                        tile_position=(64, 0),
                    )
                    pAB = psbp.tile([128, 2 * QH], BF16, tag="psb", name=f"p{p}_{t}")
                    nc.scalar.activation(
                        pAB[:], psS[:], AF.Exp, bias=lnG[:, t : t + 1], scale=1.0
                    )
                    vv = v_sb[t][:].rearrange("p (h c) -> p h c", c=DH + 1)
                    nc.tensor.matmul(
                        psC_A[:],
                        vv[:, 2 * p, :],
                        pAB[:, 0:QH],
                        start=(t == 0),
                        stop=(t == PT - 1),
                    )
                    nc.tensor.matmul(
                        psC_B[:],
                        vv[:, 2 * p + 1, :],
                        pAB[:, QH : 2 * QH],
                        start=(t == 0),
                        stop=(t == PT - 1),
                    )
                # normalize: ctx rows 0:64 divided by den row 64
                denA = normp.tile([1, QH], F32, tag="den", name=f"denA{p}")
                denB = normp.tile([1, QH], F32, tag="den", name=f"denB{p}")
                nc.vector.tensor_copy(denA[:], psC_A[64:65, :])
                nc.vector.tensor_copy(denB[:], psC_B[64:65, :])
                recA = normp.tile([1, QH], F32, tag="rec", name=f"recA{p}")
                recB = normp.tile([1, QH], F32, tag="rec", name=f"recB{p}")
                nc.vector.reciprocal_approx_fast(recA[:], denA[:])
                nc.vector.reciprocal_approx_fast(recB[:], denB[:])
                bcA = normp.tile([64, QH], F32, tag="bc", name=f"bcA{p}")
                bcB = normp.tile([64, QH], F32, tag="bc", name=f"bcB{p}")
                nc.gpsimd.partition_broadcast(bcA[:], recA[0:1, :])
                nc.gpsimd.partition_broadcast(bcB[:], recB[0:1, :])
                nc.vector.tensor_mul(ctx_sb[p][0:64, :], psC_A[0:64, :], bcA[:])
                nc.vector.tensor_mul(ctx_sb[p][64:128, :], psC_B[0:64, :], bcB[:])

            def do_oproj(n, m):
                ps = accp.tile([128, 512], F32, tag="ps", name=f"pso{n}_{m}")
                for t in range(KT):
                    nc.tensor.matmul(
                        ps[:],
                        ctx_sb[t][:, 128 * m : 128 * (m + 1)],
                        wob[:, DIM * t + 512 * n : DIM * t + 512 * (n + 1)],
                        start=(t == 0),
                        stop=False,
                    )
                nc.tensor.matmul(
                    ps[:],
                    ones[0:1, 0:128],
                    bo_sb[0:1, 512 * n : 512 * (n + 1)],
                    start=False,
                    stop=True,
                )
                os_t = osbp.tile([128, 512], F32, tag="osb", name=f"os{n}_{m}")
                nc.vector.tensor_copy(os_t[:], ps[:])
                nc.sync.dma_start(
                    out=out[128 * m : 128 * (m + 1), 512 * n : 512 * (n + 1)],
                    in_=os_t[:],
                )

            # ---- emission schedule: DMA chunks early, attention pairs
            # interleaved with remaining projection work so the ACT-bound
            # exp stream overlaps PE-bound projection matmuls.
            load_chunk(qTb, qT, QH, 0)
            load_chunk(wqb, Wq, DIM, 0)
            load_chunk(qTb, qT, QH, 1)
            load_chunk(wqb, Wq, DIM, 1)
            load_chunk(wkb, Wk, DIM, 0)
            load_chunk(ktb, kT, SEQ, 0)
            load_chunk(wkb, Wk, DIM, 1)
            load_chunk(ktb, kT, SEQ, 1)
            load_chunk(vtb, vT, SEQ, 0)
            load_chunk(wvb, Wv, DIM, 0)
            load_chunk(vtb, vT, SEQ, 1)
            load_chunk(wvb, Wv, DIM, 1)
            load_chunk(wob, Wo, DIM, 0)
            load_chunk(wob, Wo, DIM, 1)

            for j in range(KT):
                do_qproj(j)
            for j in range(4):
                do_kproj(j)
            for m in range(PT):
                do_vproj(0, m)
            do_pair(0)
            do_vproj(1, 0)
            do_vproj(1, 1)
            do_pair(1)
            do_vproj(1, 2)
            do_vproj(1, 3)
            do_pair(2)
            do_vproj(1, 4)
            do_vproj(1, 5)
            do_pair(3)
            do_vproj(1, 6)
            do_vproj(1, 7)
            do_kproj(4)
            do_pair(4)
            do_kproj(5)
            do_pair(5)
            do_kproj(6)
            do_pair(6)
            do_kproj(7)
            do_pair(7)
            for n in range(2):
                for m in range(QH // 128):
                    do_oproj(n, m)

    nc.compile()
    return nc


def kernel(
    query, key, value, mask, gauss_weight, Wq, bq, Wk, bk, Wv, bv, Wo, bo
) -> np.ndarray:
    global LAST_RESULT
    if "nc" not in _CACHED:
        _CACHED["nc"] = _build()
    nc = _CACHED["nc"]

    query = np.asarray(query, dtype=np.float32)
    key = np.asarray(key, dtype=np.float32)
    value = np.asarray(value, dtype=np.float32)
    mask = np.asarray(mask, dtype=np.int32)
    gauss_weight = np.asarray(gauss_weight, dtype=np.float32)
    shared = {
        "Wq": np.ascontiguousarray(Wq, dtype=np.float32),
        "Wk": np.ascontiguousarray(Wk, dtype=np.float32),
        "Wv": np.ascontiguousarray(Wv, dtype=np.float32),
        "Wo": np.ascontiguousarray(Wo, dtype=np.float32),
        "bq": np.ascontiguousarray(bq, dtype=np.float32),
        "bk": np.ascontiguousarray(bk, dtype=np.float32),
        "bv": np.ascontiguousarray(bv, dtype=np.float32),
        "bo": np.ascontiguousarray(bo, dtype=np.float32),
    }

    in_maps = []
    for c in range(N_CORES):
        b, r = c // 2, c % 2
        qTb = np.ascontiguousarray(query[b].T[:, QH * r : QH * (r + 1)])
        in_maps.append(
            {
                "qT": qTb,
                "kT": np.ascontiguousarray(key[b].T),
                "vT": np.ascontiguousarray(value[b].T),
                "gauss": np.ascontiguousarray(gauss_weight[b]),
                "mask": np.ascontiguousarray(mask[b]),
                **shared,
            }
        )

    res = None
    last_exc = None
    for _attempt in range(3):
        try:
            res = bass_utils.run_bass_kernel_spmd(
                nc, in_maps, core_ids=list(range(N_CORES))
            )
            break
        except Exception as e:  # transient NRT_EXEC_UNIT faults on first exec
            last_exc = e
    if res is None:
        raise last_exc
    LAST_RESULT = res

    output = np.empty((BS, SEQ, DIM), dtype=np.float32)
    for c in range(N_CORES):
        b, r = c // 2, c % 2
        output[b, QH * r : QH * (r + 1), :] = res.results[c]["out"]
    return output
